# revision 44
# baseline (speedup 1.0000x reference)
"""Llama GQA attention layer (B=1, S=2048, E=4096, H=32, HKV=8, D=128) on 8
Trainium2 NeuronCores.

Sharding: tensor-parallel over heads. Core c owns Q heads 4c..4c+3 and KV head
c (KV groups stay intact), plus the matching Wo input-dim slice. Each core
computes a full [S, E] partial of the o_proj output; the host sums the 8
partials (the "all-reduce after o_proj").

All matmul operands are bf16 (PSUM accumulation fp32); PE cost on TRN2 is
1 cycle/moving-row either way, but bf16 halves DMA + SBUF so everything stays
resident and the PE never starves.

Per-core dataflow, one pass per 512-token group g (no DRAM spills):
  QKV: six serialized 32-chunk PSUM chains in order [k, q0, q1, q2, q3, v];
    RoPE is applied right after each chain retires: the half-rotation is a PE
    matmul against a host-supplied 128x128 permutation (sign folded into the
    sin table, 1/sqrt(D) folded into q's cos/sin), then 3 DVE ops.
    v is transposed to [tok, d] via PE transpose.
  attention: flat software-pipelined loop over (head, key-tile); scoresT[k,q]
    -> exp on ScalarE -> av/den accumulate in PSUM. den uses an all-ones
    [128,128] stationary so every PSUM row holds the denominator (broadcast
    for free); causal diagonal tiles compute only live columns (c0=128j).
    Softmax has no max subtraction (scores ~ N(0,1), exp cannot overflow).
    Epilogue: reciprocal_approx_fast (DVE custom op) + one DVE mul -> ao.
  o_proj: ao (SBUF) @ Wo accumulated over the 4 local heads, per 128-token
    tile; result DMAed out bf16. Host sums the 8 partial [S, E] outputs.
"""

import sys
import types

if "/opt/trn_rl_repo" not in sys.path:
    sys.path.insert(0, "/opt/trn_rl_repo")

import numpy as np
import ml_dtypes

import concourse.bass as bass
import concourse.tile as tile
from concourse import bacc, mybir
from concourse.bass_utils import run_bass_kernel_spmd

F32 = mybir.dt.float32
BF16 = mybir.dt.bfloat16
EXP = mybir.ActivationFunctionType.Exp
BF = ml_dtypes.bfloat16

S = 2048
E = 4096
H = 32
HKV = 8
D = 128
NCORES = 8
HL = H // NCORES          # 4 local q heads per core
TG = 512                  # token group (moving-dim tile)
NG = S // TG              # 4 token groups
NE = E // 128             # 32 contraction chunks
NK = S // 128             # 16 key tiles
EB = 8                    # e-chunks per hsT DMA block
NB = NE // EB             # 4 blocks per group
NEG = -1e9

TRACE = [False]
LAST_EXEC_NS = [None]
LAST_RES = [None]

_PROGRAMS = {}


def _install_ntff_hook():
    if "antenv.axon_hooks" in sys.modules:
        return
    mod = types.ModuleType("antenv.axon_hooks")
    hook = [None]
    mod.set_axon_ntff_profile_hook = lambda h: hook.__setitem__(0, h)
    mod.get_axon_ntff_profile_hook = lambda: hook[0]
    sys.modules["antenv.axon_hooks"] = mod
    try:
        from trn_agent_boot.trn_boot import _ntff_profile_via_ctypes

        mod.set_axon_ntff_profile_hook(
            _ntff_profile_via_ctypes("/opt/axon/libaxon_pjrt.so"))
    except Exception:
        pass


def set_trace(on=True):
    if on:
        _install_ntff_hook()
    TRACE[0] = on


def _build_program(mode):
    """mode: 'causal' (skip above-diagonal key tiles, live-column diagonal),
    'full' (no mask), 'general' (additive mask streamed from DRAM).
    """
    nc = bacc.Bacc(trn_type="TRN2", target_bir_lowering=False, debug=False)

    hsT_d = nc.dram_tensor("hsT", [NG, E, TG], BF16, kind="ExternalInput").ap()
    wqT_d = nc.dram_tensor("wqT", [HL, E, D], BF16, kind="ExternalInput").ap()
    wkT_d = nc.dram_tensor("wkT", [E, D], BF16, kind="ExternalInput").ap()
    wvT_d = nc.dram_tensor("wvT", [E, D], BF16, kind="ExternalInput").ap()
    woT_d = nc.dram_tensor("woT", [HL * D, E], BF16, kind="ExternalInput").ap()
    cosq_d = nc.dram_tensor("cosq", [D, S], BF16, kind="ExternalInput").ap()
    sinq_d = nc.dram_tensor("sinq", [D, S], BF16, kind="ExternalInput").ap()
    cosk_d = nc.dram_tensor("cosk", [D, S], BF16, kind="ExternalInput").ap()
    sink_d = nc.dram_tensor("sink", [D, S], BF16, kind="ExternalInput").ap()
    pswap_d = nc.dram_tensor("pswap", [128, 128], BF16, kind="ExternalInput").ap()
    onesm_d = nc.dram_tensor("onesm", [128, 128], BF16, kind="ExternalInput").ap()
    ident_d = nc.dram_tensor("ident", [128, 128], F32, kind="ExternalInput").ap()
    if mode == "causal":
        cmask_d = nc.dram_tensor("cmask", [128, 128], F32,
                                 kind="ExternalInput").ap()
    elif mode == "general":
        maskT_d = nc.dram_tensor("maskT", [S, S], BF16, kind="ExternalInput").ap()
    outp_d = nc.dram_tensor("outp", [NK, 128, E], BF16,
                            kind="ExternalOutput").ap()

    wq_r = wqT_d.rearrange("h (ne p) f -> h p ne f", p=128)
    wk_r = wkT_d.rearrange("(ne p) f -> p ne f", p=128)
    wv_r = wvT_d.rearrange("(ne p) f -> p ne f", p=128)
    wo_r = woT_d.rearrange("(h p) e -> p h e", p=128)
    hs_r = hsT_d.rearrange("g (c p) t -> g p c t", p=128)

    with tile.TileContext(nc) as tc:
      with nc.allow_low_precision(reason="bf16 attention kernel"), \
           tc.tile_pool(name="const", bufs=1) as cp, \
           tc.tile_pool(name="persist", bufs=1) as pp, \
           tc.tile_pool(name="hstp", bufs=5) as hst_pool, \
           tc.tile_pool(name="qrop", bufs=2) as qro_pool, \
           tc.tile_pool(name="xsp", bufs=3) as xs_pool, \
           tc.tile_pool(name="t12p", bufs=2) as t12_pool, \
           tc.tile_pool(name="vsp", bufs=1) as vs_pool, \
           tc.tile_pool(name="exp", bufs=7) as ex_pool, \
           tc.tile_pool(name="rcp", bufs=2) as rc_pool, \
           tc.tile_pool(name="ostp", bufs=2) as ost_pool, \
           tc.tile_pool(name="mtp", bufs=4) as mt_pool, \
           tc.tile_pool(name="ps", bufs=1, space="PSUM") as ps:

        # --- persistent SBUF ---
        wq_sb = pp.tile([128, HL, NE, D], BF16, name="wq_sb")
        wk_sb = pp.tile([128, NE, D], BF16, name="wk_sb")
        wv_sb = pp.tile([128, NE, D], BF16, name="wv_sb")
        wo_sb = pp.tile([128, HL, E], BF16, name="wo_sb")
        cq_sb = pp.tile([128, S], BF16, name="cq_sb")
        sq_sb = pp.tile([128, S], BF16, name="sq_sb")
        ck_sb = pp.tile([128, S], BF16, name="ck_sb")
        sk_sb = pp.tile([128, S], BF16, name="sk_sb")
        krope = pp.tile([128, S], BF16, name="krope")
        vnat = pp.tile([128, NK, D], BF16, name="vnat")
        ao_all = pp.tile([128, HL, TG], BF16, name="ao_all")
        pswap = cp.tile([128, 128], BF16, name="pswap")
        ones_sb = cp.tile([128, 128], BF16, name="ones_sb")
        ident = cp.tile([128, 128], F32, name="ident")
        if mode == "causal":
            cmask = cp.tile([128, 128], F32, name="cmask")

        hst = {}

        def issue_hst(g, b):
            t = hst_pool.tile([128, EB, TG], BF16, tag="hst",
                              name=f"hst{g}_{b}")
            nc.sync.dma_start(out=t, in_=hs_r[g, :, EB * b:EB * (b + 1), :])
            hst[(g, b)] = t

        # --- startup DMAs on the sync hardware queue only (gpsimd is a slow
        # software DGE), ordered by first use under chain order
        # [k, v, q0..q3]: wk interleaved with hsT blocks, then wv, then one
        # wq head at a time (q0 starts after 1 MB), rope tables (gate only
        # the first scores, ~50us away), consts, wo last (~75us away). ---
        h0t = {}
        for b in range(NB):
            h0t[b] = hst_pool.tile([128, EB, TG], BF16, tag="hst",
                                   name=f"hst0_{b}")
            hst[(0, b)] = h0t[b]

        def hst0_piece(b, lo, hi):
            nc.sync.dma_start(out=h0t[b][:, lo:hi, :],
                              in_=hs_r[0, :, EB * b + lo:EB * b + hi, :])

        nc.sync.dma_start(out=wk_sb[:, 0:4, :], in_=wk_r[:, 0:4, :])
        nc.sync.dma_start(out=wv_sb[:, 0:4, :], in_=wv_r[:, 0:4, :])
        hst0_piece(0, 0, 4)
        nc.sync.dma_start(out=pswap, in_=pswap_d)
        nc.sync.dma_start(out=ident, in_=ident_d)
        nc.sync.dma_start(out=wk_sb[:, 4:12, :], in_=wk_r[:, 4:12, :])
        nc.sync.dma_start(out=wv_sb[:, 4:12, :], in_=wv_r[:, 4:12, :])
        hst0_piece(0, 4, 8)
        hst0_piece(1, 0, 4)
        nc.sync.dma_start(out=wk_sb[:, 12:20, :], in_=wk_r[:, 12:20, :])
        nc.sync.dma_start(out=wv_sb[:, 12:20, :], in_=wv_r[:, 12:20, :])
        hst0_piece(1, 4, 8)
        hst0_piece(2, 0, 4)
        nc.sync.dma_start(out=wk_sb[:, 20:32, :], in_=wk_r[:, 20:32, :])
        nc.sync.dma_start(out=wv_sb[:, 20:32, :], in_=wv_r[:, 20:32, :])
        hst0_piece(2, 4, 8)
        hst0_piece(3, 0, 4)
        hst0_piece(3, 4, 8)
        nc.sync.dma_start(out=wq_sb[:, 0], in_=wq_r[0])
        nc.sync.dma_start(out=ck_sb, in_=cosk_d)
        nc.sync.dma_start(out=sk_sb, in_=sink_d)
        nc.sync.dma_start(out=wq_sb[:, 1], in_=wq_r[1])
        nc.sync.dma_start(out=cq_sb, in_=cosq_d)
        nc.sync.dma_start(out=sq_sb, in_=sinq_d)
        nc.sync.dma_start(out=wq_sb[:, 2], in_=wq_r[2])
        nc.sync.dma_start(out=wq_sb[:, 3], in_=wq_r[3])
        nc.sync.dma_start(out=ones_sb, in_=onesm_d)
        if mode == "causal":
            nc.sync.dma_start(out=cmask, in_=cmask_d)
        nc.sync.dma_start(out=wo_sb[:, :, :E // 2], in_=wo_r[:, :, :E // 2])
        nc.sync.dma_start(out=wo_sb[:, :, E // 2:], in_=wo_r[:, :, E // 2:])

        prot = [0]   # rotation counters: P0-P2 (chains / scores)
        rrot = [0]   # R0-R1 (rope swaps / v transposes / av)

        def chain_matmuls(g, wsel, tag):
            x_ps = ps.tile([128, TG], F32, tag=tag, name=f"ch_{tag}_{g}")
            for e in range(NE):
                nc.tensor.matmul(x_ps, wsel(e),
                                 hst[(g, e // EB)][:, e % EB, :],
                                 start=(e == 0), stop=(e == NE - 1))
            return x_ps

        def rope(g, x_ps, cos_sb, sin_sb, out_ap):
            t0 = g * TG
            xs = xs_pool.tile([128, TG], BF16, tag="xs", name="xs")
            nc.scalar.copy(out=xs, in_=x_ps)
            # half-rotation via PE permutation matmul (sign folded into sin)
            sw = ps.tile([128, TG], F32, tag=f"R{rrot[0] % 2}", name="sw")
            rrot[0] += 1
            nc.tensor.matmul(sw, pswap, xs, start=True, stop=True)
            t1 = t12_pool.tile([128, TG], BF16, tag="t1", name="t1")
            nc.vector.tensor_mul(t1, xs, cos_sb[:, t0:t0 + TG])
            t2 = t12_pool.tile([128, TG], BF16, tag="t2", name="t2")
            nc.vector.tensor_mul(t2, sw, sin_sb[:, t0:t0 + TG])
            nc.vector.tensor_add(out_ap, t1, t2)

        for g in range(NG):
            t0 = g * TG
            qro = qro_pool.tile([128, HL, TG], BF16, tag="qro", name=f"qro{g}")

            # ---- QKV: k and v chains interleaved chunk-by-chunk so the
            # startup DMA (350 GB/s) keeps up with stationary consumption;
            # q chains after (their 4 MB of wq streams in meanwhile). ----
            k_ps = ps.tile([128, TG], F32, tag="P0", name=f"k_ps{g}")
            v_ps = ps.tile([128, TG], F32, tag="P1", name=f"v_ps{g}")
            for e in range(NE):
                hs_e = hst[(g, e // EB)][:, e % EB, :]
                nc.tensor.matmul(k_ps, wk_sb[:, e, :], hs_e,
                                 start=(e == 0), stop=(e == NE - 1))
                nc.tensor.matmul(v_ps, wv_sb[:, e, :], hs_e,
                                 start=(e == 0), stop=(e == NE - 1))
            rope(g, k_ps, ck_sb, sk_sb, krope[:, t0:t0 + TG])
            vs = vs_pool.tile([128, TG], F32, tag="vs", name="vs")
            nc.scalar.copy(out=vs, in_=v_ps)
            for j in range(4):
                tr = ps.tile([128, 128], F32, tag=f"R{rrot[0] % 2}", name="tr")
                rrot[0] += 1
                nc.tensor.transpose(tr, vs[:, 128 * j:128 * (j + 1)], ident)
                nc.vector.tensor_copy(vnat[:, 4 * g + j, :], tr)
            for f in range(HL):
                q_ps = chain_matmuls(g, lambda e, f=f: wq_sb[:, f, e, :],
                                     ["P2", "P0", "P1", "P2"][f])
                rope(g, q_ps, cq_sb, sq_sb, qro[:, f, :])

            # ---- attention: flat pipelined loop over (head, key tile) ----
            nk = 4 * g + 4 if mode == "causal" else NK
            av_ps = {}
            den_ps = {}
            queue = []
            srot = [0]

            def emit_front(h, ki):
                c0 = 128 * (ki - 4 * g) if (mode == "causal" and ki >= 4 * g) \
                    else 0
                sp = ps.tile([128, TG], F32,
                             tag=["P0", "P1", "P2", "O0"][srot[0] % 4],
                             name="sp")
                srot[0] += 1
                nc.tensor.matmul(sp[:, c0:], krope[:, 128 * ki:128 * (ki + 1)],
                                 qro[:, h, c0:], start=True, stop=True)
                if mode == "causal" and ki >= 4 * g:
                    nc.vector.tensor_add(sp[:, c0:c0 + 128],
                                         sp[:, c0:c0 + 128], cmask)
                elif mode == "general":
                    mt = mt_pool.tile([128, TG], BF16, tag="mt", name="mt")
                    nc.sync.dma_start(
                        out=mt,
                        in_=maskT_d[128 * ki:128 * (ki + 1), t0:t0 + TG])
                    nc.vector.tensor_add(sp, sp, mt)
                ex = ex_pool.tile([128, TG], BF16, tag="ex", name="ex")
                nc.scalar.activation(out=ex[:, c0:], in_=sp[:, c0:], func=EXP)
                return (h, ki, c0, ex)

            def drain_one():
                h, ki, c0, ex = queue.pop(0)
                if ki == 0:
                    av_ps[h] = ps.tile([128, TG], F32, tag=f"R{h % 2}",
                                       name=f"av{h}")
                    den_ps[h] = ps.tile([128, TG], F32, tag=f"D{h % 2}",
                                        name=f"den{h}")
                last = (ki == nk - 1)
                nc.tensor.matmul(av_ps[h][:, c0:], vnat[:, ki, :], ex[:, c0:],
                                 start=(ki == 0), stop=last,
                                 skip_group_check=True)
                nc.tensor.matmul(den_ps[h][:, c0:], ones_sb, ex[:, c0:],
                                 start=(ki == 0), stop=last,
                                 skip_group_check=True)
                if last:
                    rc = rc_pool.tile([128, TG], F32, tag="rc", name="rc")
                    nc.vector.reciprocal_approx_fast(out=rc, in_=den_ps[h])
                    nc.vector.tensor_mul(ao_all[:, h, :], av_ps[h], rc)

            nev = 0
            for h in range(HL):
                for ki in range(nk):
                    queue.append(emit_front(h, ki))
                    nev += 1
                    # prefetch next group's hsT right away (buffers are free
                    # and the sync queue is otherwise idle during attention)
                    if g < NG - 1 and nev <= NB:
                        issue_hst(g + 1, nev - 1)
                    if len(queue) >= 4:
                        drain_one()
            while queue:
                drain_one()

            # ---- o_proj for this group's 512 tokens. h outer so the ao
            # stationary is loaded once per (ti, h): 4 LDWEIGHTS per token
            # tile instead of 32, with all 8 PSUM banks accumulating. ----
            OTAGS = ["P0", "P1", "P2", "O0", "R0", "R1", "D0", "D1"]
            for ti in range(4):
                ost = ost_pool.tile([128, E], BF16, tag="ost", name=f"ost{ti}")
                last = (g == NG - 1 and ti == 3)

                def drain_eg(eg):
                    if eg % 2:
                        nc.vector.tensor_copy(ost[:, TG * eg:TG * (eg + 1)],
                                              ops[eg])
                    else:
                        nc.scalar.copy(out=ost[:, TG * eg:TG * (eg + 1)],
                                       in_=ops[eg])
                    if eg == 3:
                        nc.sync.dma_start(out=outp_d[4 * g + ti][:, :E // 2],
                                          in_=ost[:, :E // 2])
                    elif last and eg == 5:
                        nc.sync.dma_start(
                            out=outp_d[4 * g + ti][:, E // 2:3 * E // 4],
                            in_=ost[:, E // 2:3 * E // 4])

                ops = {}
                for h in range(HL):
                    for eg in range(E // TG):
                        if h == 0:
                            ops[eg] = ps.tile([128, TG], F32, tag=OTAGS[eg],
                                              name=f"op{eg}")
                        nc.tensor.matmul(
                            ops[eg], ao_all[:, h, 128 * ti:128 * (ti + 1)],
                            wo_sb[:, h, TG * eg:TG * (eg + 1)],
                            start=(h == 0), stop=(h == HL - 1))
                        if last and h == HL - 1:
                            drain_eg(eg)   # hide copies under the h3 sweep
                if not last:
                    for eg in range(E // TG):
                        drain_eg(eg)
                    nc.sync.dma_start(out=outp_d[4 * g + ti][:, E // 2:],
                                      in_=ost[:, E // 2:])
                else:
                    nc.sync.dma_start(out=outp_d[4 * g + ti][:, 3 * E // 4:],
                                      in_=ost[:, 3 * E // 4:])

    nc.compile()
    return nc


_CONSTS = None


def _consts():
    global _CONSTS
    if _CONSTS is None:
        kp = np.arange(128)[:, None]
        qc = np.arange(128)[None, :]
        cmask = np.where(qc >= kp, 0.0, NEG).astype(np.float32)
        pswap = np.roll(np.eye(128, dtype=np.float32), 64, axis=0).astype(BF)
        ones = np.ones((128, 128), dtype=BF)
        ident = np.eye(128, dtype=np.float32)
        _CONSTS = (cmask, pswap, ones, ident)
    return _CONSTS


def _rope_tables(position_ids):
    pos = np.asarray(position_ids[0]).astype(np.float32)          # [S]
    inv_freq = (1.0 / (10000.0 ** (np.arange(0, D, 2, dtype=np.float32) / D)))
    freqs = pos[:, None] * inv_freq[None, :]                      # [S, 64]
    emb = np.concatenate([freqs, freqs], axis=1)                  # [S, 128]
    cosT = np.cos(emb).T.astype(np.float32).copy()                # [128, S]
    sinT = np.sin(emb).T.astype(np.float32)
    sinflipT = np.concatenate([-sinT[:64], sinT[64:]], axis=0)
    sc = np.float32(1.0 / np.sqrt(D))
    return ((cosT * sc).astype(BF), (sinflipT * sc).astype(BF),
            cosT.astype(BF), sinflipT.astype(BF))


def kernel(hidden_states, position_ids, attention_mask, Wq, Wk, Wv, Wo):
    hidden_states = np.asarray(hidden_states)
    B = hidden_states.shape[0]
    assert hidden_states.shape == (B, S, E), hidden_states.shape
    assert B == 1

    mask = np.asarray(attention_mask, dtype=np.float32)[0, 0]
    if not mask.any():
        mode = "full"
    elif np.array_equal(mask, np.triu(np.full((S, S), NEG, dtype=np.float32), 1)):
        mode = "causal"
    else:
        mode = "general"

    if mode not in _PROGRAMS:
        _PROGRAMS[mode] = _build_program(mode)
    nc = _PROGRAMS[mode]

    hs = np.asarray(hidden_states[0], dtype=np.float32)
    # [E, S] -> group-major [NG, E, TG], bf16
    hsT = np.ascontiguousarray(
        hs.T.reshape(E, NG, TG).transpose(1, 0, 2)).astype(BF)
    cosq, sinq, cosk, sink = _rope_tables(np.asarray(position_ids))
    Wq = np.asarray(Wq, dtype=np.float32)
    Wk = np.asarray(Wk, dtype=np.float32)
    Wv = np.asarray(Wv, dtype=np.float32)
    Wo = np.asarray(Wo, dtype=np.float32)
    cmask, pswap, ones, ident = _consts()

    in_maps = []
    for c in range(NCORES):
        m = {
            "hsT": hsT,
            "wqT": np.ascontiguousarray(
                Wq[512 * c:512 * (c + 1), :].T.reshape(E, HL, D)
                .transpose(1, 0, 2)).astype(BF),
            "wkT": np.ascontiguousarray(Wk[128 * c:128 * (c + 1), :].T).astype(BF),
            "wvT": np.ascontiguousarray(Wv[128 * c:128 * (c + 1), :].T).astype(BF),
            "woT": np.ascontiguousarray(Wo[:, 512 * c:512 * (c + 1)].T).astype(BF),
            "cosq": cosq, "sinq": sinq, "cosk": cosk, "sink": sink,
            "pswap": pswap, "onesm": ones, "ident": ident,
        }
        if mode == "causal":
            m["cmask"] = cmask
        elif mode == "general":
            m["maskT"] = np.ascontiguousarray(mask.T).astype(BF)
        in_maps.append(m)

    res = run_bass_kernel_spmd(nc, in_maps, core_ids=list(range(NCORES)),
                               trace=TRACE[0])
    LAST_EXEC_NS[0] = res.exec_time_ns
    LAST_RES[0] = res

    acc = np.zeros((NK, 128, E), dtype=np.float32)
    for c in range(NCORES):
        acc += res.results[c]["outp"].astype(np.float32)
    out = acc.reshape(S, E)
    return out[None, :, :]
